# revision 10
# baseline (speedup 1.0000x reference)
"""Divergence-free kernel (N=2048, M=2048, D=16) on 8 Trainium2 NeuronCores.

Math
----
ls = softplus(uls); var = softplus(uv); l2 = 1/ls^2; S = sum(l2); w = l2^2-S*l2
E[n,m]   = exp(G1[n,m] - 0.5*X2s_l[m] - 0.5*Xs_l[n])      G1 = sum_d l2 X X2
P[n,m]   = -2var*Gw[n,m] + var*v_w[m] + var*u_w[n] + (D-1)*S*var
out[n,m] = E * P
with Xs_l/X2s_l the l2-weighted squared rows and u_w/v_w the w-weighted ones.

All parameter-dependent operand prep happens on the HOST (softplus is over 17
scalars).  The device kernel is only:
  2 input DMAs (bf16 matmul operands; first covers the first super-tile so
  the PE can start ~0.5us earlier) + 1 tiny f32 bias DMA
  16 matmuls  (K=18: 16 data rows + 1 const row per plane; bf16 streams
  rows at twice the fp32r rate through the PE)
  ACT exps (per-partition bias), scalar_tensor_tensor on DVE; 1024-wide for
  the first three super-tiles, 512-wide for the last one so the final
  ACT->stt->DMA serial tail is short.
  out DMAs in bf16 (halves HBM write traffic; host upcasts)

Sharding: rows of X split across 8 cores (256 rows each); X2 + params
replicated.
"""

import os
import sys

import numpy as np

for _p in ("/opt/trn_rl_repo", "/root/.axon_site/_ro/trn_rl_repo"):
    if os.path.isdir(_p) and _p not in sys.path:
        sys.path.insert(0, _p)

import concourse.bass as bass
import concourse.bacc as bacc
import concourse.tile as tile
from concourse import mybir
from concourse.bass_utils import run_bass_kernel_spmd

N, M, D = 2048, 2048, 16
NCORES = 8
NLOC = N // NCORES          # 256 rows per core
NT = NLOC // 128            # 2 n-tiles of 128 rows
# contraction rows: 16 data + 1 const (E) + 1 const (R), optionally padded
# to 32 (bf16 weight loads at odd partition counts have crashed the PE).
KP = int(os.environ.get("DFK_KP", "18"))
MTILE = 512                 # matmul free dim (one PSUM bank)
# rblob layout: LTE(256) | LTR(256) | R(2048)
RW = 2 * NLOC + M
R0 = 2 * NLOC               # column where R starts
SPLIT = R0 + 512            # first DMA covers LTE+LTR+R[:, :512]

F32 = mybir.dt.float32
F32R = mybir.dt.float32r
BF16 = mybir.dt.bfloat16
# matmul operand dtype: bf16 (1) or fp32r (0)
USE_BF16 = bool(int(os.environ.get("DFK_BF16", "1")))
DT_MM = BF16 if USE_BF16 else F32R
AF = mybir.ActivationFunctionType
ALU = mybir.AluOpType


def build_nc() -> bass.Bass:
    # Bacc (not raw Bass): its compile() legalizes sync waits for TRN2's
    # one-wait-per-instruction ISA limit.
    nc = bacc.Bacc("TRN2", target_bir_lowering=False)

    rblob_d = nc.dram_tensor("rblob", [KP, RW], DT_MM, kind="ExternalInput")
    bias_d = nc.dram_tensor("bias", [128, 2 * NT], F32, kind="ExternalInput")
    out_d = nc.dram_tensor("out", [NLOC, M], BF16, kind="ExternalOutput")

    with tile.TileContext(nc) as tc:
        with (
            tc.tile_pool(name="const", bufs=1) as cp,
            tc.tile_pool(name="pe", bufs=2, space=bass.MemorySpace.PSUM) as pep,
            tc.tile_pool(name="pr", bufs=2, space=bass.MemorySpace.PSUM) as prp,
            tc.tile_pool(name="eb", bufs=4) as ebp,
            tc.tile_pool(name="osb", bufs=4) as osp,
        ):
            # keep the Exp table resident before the first activation; queued
            # first on ACT so it overlaps the input DMA.
            ld = mybir.InstLoadActFuncSet(
                name=nc.get_next_instruction_name(),
                ins=[],
                outs=[],
                act_func_set_id=0,  # exp_and_others
            )
            ld.engine = nc.scalar.engine
            nc.scalar.add_instruction(ld)

            RT = cp.tile([KP, RW], DT_MM)
            nc.sync.dma_start(out=RT[:, 0:SPLIT], in_=rblob_d[:, 0:SPLIT])
            nc.sync.dma_start(out=RT[:, SPLIT:RW], in_=rblob_d[:, SPLIT:RW])
            BT = cp.tile([128, 2 * NT], F32)
            nc.gpsimd.dma_start(out=BT[:], in_=bias_d[:, :])

            # matmul free dim is capped at 512 output elements per
            # instruction (ISA s3d3_mm_num_elements).
            MMW = int(os.environ.get("DFK_MMW", "512"))
            # super-tile schedule: 1024 columns each.  ACT runs 512-wide
            # chunks into halves of one eb tile (each chunk starts right
            # after its E-matmul lands); stt is 1024-wide except on the
            # final super-tile (2x512 so the final ACT -> stt -> out-DMA
            # serial tail is short).
            sched = [(0, 0), (0, 1), (1, 0), (1, 1)]
            for si, (i, jh) in enumerate(sched):
                lte = RT[:, i * 128 : (i + 1) * 128]
                ltr = RT[:, NLOC + i * 128 : NLOC + (i + 1) * 128]
                pe_ = pep.tile([128, 1024], F32, tag="pe")
                pr_ = prp.tile([128, 1024], F32, tag="pr")
                for jl in range(1024 // MMW):
                    cs = slice(R0 + jh * 1024 + jl * MMW, R0 + jh * 1024 + (jl + 1) * MMW)
                    nc.tensor.matmul(pe_[:, jl * MMW : (jl + 1) * MMW], lte, RT[:, cs])
                for jl in range(1024 // MMW):
                    cs = slice(R0 + jh * 1024 + jl * MMW, R0 + jh * 1024 + (jl + 1) * MMW)
                    nc.tensor.matmul(pr_[:, jl * MMW : (jl + 1) * MMW], ltr, RT[:, cs])
                eb = ebp.tile([128, 1024], F32, tag="eb")
                for c in range(2):
                    ls_ = slice(c * 512, (c + 1) * 512)
                    nc.scalar.activation(
                        out=eb[:, ls_],
                        in_=pe_[:, ls_],
                        func=AF.Exp,
                        bias=BT[:, i : i + 1],
                        scale=1.0,
                    )
                chunks = 1 if si < 3 else 2
                cw = 1024 // chunks
                for c in range(chunks):
                    ls_ = slice(c * cw, (c + 1) * cw)
                    osb = osp.tile([128, cw], BF16, tag=f"osb{chunks}")
                    nc.vector.scalar_tensor_tensor(
                        osb[:],
                        in0=pr_[:, ls_],
                        scalar=BT[:, NT + i : NT + i + 1],
                        in1=eb[:, ls_],
                        op0=ALU.add,
                        op1=ALU.mult,
                    )
                    hs = slice(jh * 1024 + c * cw, jh * 1024 + (c + 1) * cw)
                    nc.sync.dma_start(
                        out=out_d[i * 128 : (i + 1) * 128, hs], in_=osb[:]
                    )

    nc.finalize()
    return nc


_NC_CACHE: bass.Bass | None = None


def _get_nc() -> bass.Bass:
    global _NC_CACHE
    if _NC_CACHE is None:
        _NC_CACHE = build_nc()
    return _NC_CACHE


def make_in_maps(X, X2, uls, uv):
    import ml_dtypes

    X = np.asarray(X, dtype=np.float32).astype(np.float64)
    X2 = np.asarray(X2, dtype=np.float32).astype(np.float64)
    uls = np.asarray(uls, dtype=np.float32).reshape(D).astype(np.float64)
    uv = np.asarray(uv, dtype=np.float32).reshape(1).astype(np.float64)

    ls = np.log1p(np.exp(uls))
    var = float(np.log1p(np.exp(uv))[0])
    l2 = 1.0 / (ls * ls)
    S = float(l2.sum())
    w = l2 * l2 - S * l2

    X2T = X2.T                                       # (16, 2048)
    X2sq = X2T * X2T
    X2s_l = (l2[:, None] * X2sq).sum(0)              # (2048,)
    v_w = (w[:, None] * X2sq).sum(0)                 # (2048,)
    R = np.concatenate(
        [X2T, (-0.5 * X2s_l)[None, :], (var * v_w)[None, :]], axis=0
    )                                                # (18, 2048)

    in_maps = []
    for c in range(NCORES):
        xs = X[c * NLOC : (c + 1) * NLOC]            # (256, 16)
        lte = np.concatenate(
            [l2[:, None] * xs.T, np.ones((1, NLOC)), np.zeros((1, NLOC))], axis=0
        )                                            # (18, 256)
        ltr = np.concatenate(
            [(-2.0 * var * w)[:, None] * xs.T, np.zeros((1, NLOC)), np.ones((1, NLOC))],
            axis=0,
        )
        blob64 = np.ascontiguousarray(np.concatenate([lte, ltr, R], axis=1))
        if KP > 18:
            blob64 = np.concatenate(
                [blob64, np.zeros((KP - 18, RW))], axis=0
            )
        if USE_BF16:
            rblob = blob64.astype(np.float32).astype(ml_dtypes.bfloat16)
        else:
            b = blob64.astype(np.float32).view(np.uint32)
            b = (((b + ((b >> 12) & 1) + 0x7FF) >> 12) << 12).view(np.float32)
            rblob = np.ascontiguousarray(b)  # fp32r-exact f32 bits

        xsq = xs * xs
        biasE = -0.5 * (l2[None, :] * xsq).sum(1)    # (256,)
        cR = var * (w[None, :] * xsq).sum(1) + (D - 1) * S * var
        bias = np.empty((128, 2 * NT), dtype=np.float32)
        for i in range(NT):
            bias[:, i] = biasE[i * 128 : (i + 1) * 128]
            bias[:, NT + i] = cR[i * 128 : (i + 1) * 128]
        in_maps.append({"rblob": rblob, "bias": bias})
    return in_maps


def run(X, X2, uls, uv, trace: bool = False, **kw):
    nc = _get_nc()
    in_maps = make_in_maps(X, X2, uls, uv)
    res = run_bass_kernel_spmd(nc, in_maps, list(range(NCORES)), trace=trace, **kw)
    out = np.concatenate(
        [np.asarray(res.results[c]["out"]).astype(np.float32) for c in range(NCORES)],
        axis=0,
    )
    return out, res


def kernel(X, X2, uls, uv):
    out, _ = run(X, X2, uls, uv, trace=False)
    return out


if __name__ == "__main__":
    nc = build_nc()
    print("built ok")


# revision 12
# speedup vs baseline: 1.0209x; 1.0209x over previous
"""Divergence-free kernel (N=2048, M=2048, D=16) on 8 Trainium2 NeuronCores.

Math
----
ls = softplus(uls); var = softplus(uv); l2 = 1/ls^2; S = sum(l2); w = l2^2-S*l2
E[n,m]   = exp(G1[n,m] - 0.5*X2s_l[m] - 0.5*Xs_l[n])      G1 = sum_d l2 X X2
P[n,m]   = -2var*Gw[n,m] + var*v_w[m] + var*u_w[n] + (D-1)*S*var
out[n,m] = E * P
with Xs_l/X2s_l the l2-weighted squared rows and u_w/v_w the w-weighted ones.

All parameter-dependent operand prep happens on the HOST (softplus is over 17
scalars).  The device kernel is only:
  2 input DMAs (bf16 matmul operands; first covers the first super-tile so
  the PE can start ~0.5us earlier) + 1 tiny f32 bias DMA
  16 matmuls  (K=18: 16 data rows + 1 const row per plane; bf16 streams
  rows at twice the fp32r rate through the PE)
  ACT exps (per-partition bias), scalar_tensor_tensor on DVE; 1024-wide for
  the first three super-tiles, 512-wide for the last one so the final
  ACT->stt->DMA serial tail is short.
  out DMAs in bf16 (halves HBM write traffic; host upcasts)

Sharding: rows of X split across 8 cores (256 rows each); X2 + params
replicated.
"""

import os
import sys

import numpy as np

for _p in ("/opt/trn_rl_repo", "/root/.axon_site/_ro/trn_rl_repo"):
    if os.path.isdir(_p) and _p not in sys.path:
        sys.path.insert(0, _p)

import concourse.bass as bass
import concourse.bacc as bacc
import concourse.tile as tile
from concourse import mybir
from concourse.bass_utils import run_bass_kernel_spmd

N, M, D = 2048, 2048, 16
NCORES = 8
NLOC = N // NCORES          # 256 rows per core
NT = NLOC // 128            # 2 n-tiles of 128 rows
# contraction rows: 16 data + 1 const (E) + 1 const (R), optionally padded
# to 32 (bf16 weight loads at odd partition counts have crashed the PE).
KP = int(os.environ.get("DFK_KP", "18"))
MTILE = 512                 # matmul free dim (one PSUM bank)
# rblob layout: LTE(256) | LTR(256) | R(2048)
RW = 2 * NLOC + M
R0 = 2 * NLOC               # column where R starts
SPLIT = R0 + 512            # first DMA covers LTE+LTR+R[:, :512]

F32 = mybir.dt.float32
F32R = mybir.dt.float32r
BF16 = mybir.dt.bfloat16
# matmul operand dtype: bf16 (1) or fp32r (0)
USE_BF16 = bool(int(os.environ.get("DFK_BF16", "1")))
DT_MM = BF16 if USE_BF16 else F32R
AF = mybir.ActivationFunctionType
ALU = mybir.AluOpType


def build_nc() -> bass.Bass:
    # Bacc (not raw Bass): its compile() legalizes sync waits for TRN2's
    # one-wait-per-instruction ISA limit.
    nc = bacc.Bacc("TRN2", target_bir_lowering=False)

    rblob_d = nc.dram_tensor("rblob", [KP, RW], DT_MM, kind="ExternalInput")
    bias_d = nc.dram_tensor("bias", [128, 2 * NT], F32, kind="ExternalInput")
    out_d = nc.dram_tensor("out", [NLOC, M], BF16, kind="ExternalOutput")

    with tile.TileContext(nc) as tc:
        with (
            tc.tile_pool(name="const", bufs=1) as cp,
            tc.tile_pool(name="pe", bufs=2, space=bass.MemorySpace.PSUM) as pep,
            tc.tile_pool(name="pr", bufs=2, space=bass.MemorySpace.PSUM) as prp,
            tc.tile_pool(name="eb", bufs=4) as ebp,
            tc.tile_pool(name="osb", bufs=4) as osp,
        ):
            # keep the Exp table resident before the first activation; queued
            # first on ACT so it overlaps the input DMA.
            ld = mybir.InstLoadActFuncSet(
                name=nc.get_next_instruction_name(),
                ins=[],
                outs=[],
                act_func_set_id=0,  # exp_and_others
            )
            ld.engine = nc.scalar.engine
            nc.scalar.add_instruction(ld)

            RT = cp.tile([KP, RW], DT_MM)
            nc.sync.dma_start(out=RT[:, 0:SPLIT], in_=rblob_d[:, 0:SPLIT])
            nc.sync.dma_start(out=RT[:, SPLIT:RW], in_=rblob_d[:, SPLIT:RW])
            BT = cp.tile([128, 2 * NT], F32)
            nc.gpsimd.dma_start(out=BT[:], in_=bias_d[:, :])

            # matmul free dim is capped at 512 output elements per
            # instruction (ISA s3d3_mm_num_elements).
            #
            # Schedule: 4 super-tiles of (128 rows x 1024 cols).  First half
            # interleaves planes per super-tile (E,E,R,R) so the DVE chain
            # starts early; second half hoists the E-matmuls of both
            # remaining super-tiles ahead of their R-matmuls so the final
            # exp is ready well before the last R-matmul lands (the tail is
            # then gated by DVE + the small final DMA, not ACT).
            # PSUM: pe pool 2x2 banks + pr pool 2x2 banks = all 8 banks.
            sched = [(0, 0), (0, 1), (1, 0), (1, 1)]
            tiles = {}
            for si, (i, jh) in enumerate(sched):
                pe_t = pep.tile([128, 1024], F32, tag="pe", name=f"pe{si}")
                pr_t = prp.tile([128, 1024], F32, tag="pr", name=f"pr{si}")
                tiles[si] = (pe_t, pr_t)

            def mm(si, plane):
                i, jh = sched[si]
                lt = (
                    RT[:, i * 128 : (i + 1) * 128]
                    if plane == 0
                    else RT[:, NLOC + i * 128 : NLOC + (i + 1) * 128]
                )
                dst = tiles[si][plane]
                for jl in range(2):
                    cs = slice(
                        R0 + jh * 1024 + jl * MTILE, R0 + jh * 1024 + (jl + 1) * MTILE
                    )
                    nc.tensor.matmul(dst[:, jl * MTILE : (jl + 1) * MTILE], lt, RT[:, cs])

            def postproc(si, act_chunks, stt_chunks):
                i, jh = sched[si]
                pe_, pr_ = tiles[si]
                eb = ebp.tile([128, 1024], F32, tag="eb")
                cw = 1024 // act_chunks
                for c in range(act_chunks):
                    ls_ = slice(c * cw, (c + 1) * cw)
                    nc.scalar.activation(
                        out=eb[:, ls_],
                        in_=pe_[:, ls_],
                        func=AF.Exp,
                        bias=BT[:, i : i + 1],
                        scale=1.0,
                    )
                cw = 1024 // stt_chunks
                for c in range(stt_chunks):
                    ls_ = slice(c * cw, (c + 1) * cw)
                    osb = osp.tile([128, cw], BF16, tag=f"osb{stt_chunks}")
                    nc.vector.scalar_tensor_tensor(
                        osb[:],
                        in0=pr_[:, ls_],
                        scalar=BT[:, NT + i : NT + i + 1],
                        in1=eb[:, ls_],
                        op0=ALU.add,
                        op1=ALU.mult,
                    )
                    hs = slice(jh * 1024 + c * cw, jh * 1024 + (c + 1) * cw)
                    nc.sync.dma_start(
                        out=out_d[i * 128 : (i + 1) * 128, hs], in_=osb[:]
                    )

            mm(0, 0); mm(0, 1); postproc(0, 1, 1)
            mm(1, 0); mm(1, 1); postproc(1, 1, 1)
            mm(2, 0); mm(3, 0)
            mm(2, 1); postproc(2, 1, 1)
            mm(3, 1); postproc(3, 2, 2)

    nc.finalize()
    return nc


_NC_CACHE: bass.Bass | None = None


def _get_nc() -> bass.Bass:
    global _NC_CACHE
    if _NC_CACHE is None:
        _NC_CACHE = build_nc()
    return _NC_CACHE


def make_in_maps(X, X2, uls, uv):
    import ml_dtypes

    X = np.asarray(X, dtype=np.float32).astype(np.float64)
    X2 = np.asarray(X2, dtype=np.float32).astype(np.float64)
    uls = np.asarray(uls, dtype=np.float32).reshape(D).astype(np.float64)
    uv = np.asarray(uv, dtype=np.float32).reshape(1).astype(np.float64)

    ls = np.log1p(np.exp(uls))
    var = float(np.log1p(np.exp(uv))[0])
    l2 = 1.0 / (ls * ls)
    S = float(l2.sum())
    w = l2 * l2 - S * l2

    X2T = X2.T                                       # (16, 2048)
    X2sq = X2T * X2T
    X2s_l = (l2[:, None] * X2sq).sum(0)              # (2048,)
    v_w = (w[:, None] * X2sq).sum(0)                 # (2048,)
    R = np.concatenate(
        [X2T, (-0.5 * X2s_l)[None, :], (var * v_w)[None, :]], axis=0
    )                                                # (18, 2048)

    in_maps = []
    for c in range(NCORES):
        xs = X[c * NLOC : (c + 1) * NLOC]            # (256, 16)
        lte = np.concatenate(
            [l2[:, None] * xs.T, np.ones((1, NLOC)), np.zeros((1, NLOC))], axis=0
        )                                            # (18, 256)
        ltr = np.concatenate(
            [(-2.0 * var * w)[:, None] * xs.T, np.zeros((1, NLOC)), np.ones((1, NLOC))],
            axis=0,
        )
        blob64 = np.ascontiguousarray(np.concatenate([lte, ltr, R], axis=1))
        if KP > 18:
            blob64 = np.concatenate(
                [blob64, np.zeros((KP - 18, RW))], axis=0
            )
        if USE_BF16:
            rblob = blob64.astype(np.float32).astype(ml_dtypes.bfloat16)
        else:
            b = blob64.astype(np.float32).view(np.uint32)
            b = (((b + ((b >> 12) & 1) + 0x7FF) >> 12) << 12).view(np.float32)
            rblob = np.ascontiguousarray(b)  # fp32r-exact f32 bits

        xsq = xs * xs
        biasE = -0.5 * (l2[None, :] * xsq).sum(1)    # (256,)
        cR = var * (w[None, :] * xsq).sum(1) + (D - 1) * S * var
        bias = np.empty((128, 2 * NT), dtype=np.float32)
        for i in range(NT):
            bias[:, i] = biasE[i * 128 : (i + 1) * 128]
            bias[:, NT + i] = cR[i * 128 : (i + 1) * 128]
        in_maps.append({"rblob": rblob, "bias": bias})
    return in_maps


def run(X, X2, uls, uv, trace: bool = False, **kw):
    nc = _get_nc()
    in_maps = make_in_maps(X, X2, uls, uv)
    res = run_bass_kernel_spmd(nc, in_maps, list(range(NCORES)), trace=trace, **kw)
    out = np.concatenate(
        [np.asarray(res.results[c]["out"]).astype(np.float32) for c in range(NCORES)],
        axis=0,
    )
    return out, res


def kernel(X, X2, uls, uv):
    out, _ = run(X, X2, uls, uv, trace=False)
    return out


if __name__ == "__main__":
    nc = build_nc()
    print("built ok")
